# revision 1
# baseline (speedup 1.0000x reference)
"""CrossStageMoE kernel for 8 trn2 NeuronCores.

Reference computation (per batch b):
    g[b]  = softmax(MLP(mean_n x[b])),            [E=8]
    Wb[b] = sum_e g[b,e] * We[e],                 [O=1024, C=1024]
    y     = x[b] @ Wb[b].T + g[b] @ be            (for x and x_ir)

Sharding: core k -> (b = k % 4, h = k // 4).  Each core handles one batch and
one 512-wide half of the output dim O, for BOTH token tensors (x, x_ir).
Gates are recomputed redundantly on both cores of a batch (cheap, and both
already hold x[b] transposed for the main matmul).

On-core pipeline (all matmul compute in bf16, fp32 PSUM accumulate):
  1. gx = mean_n x[b]            DVE free-axis reductions over xT tiles
  2. hT = relu(gw1 @ gx + gb1)   PE matvecs, h kept on partitions
  3. logits -> softmax -> g      tiny free-axis ops on [1, 8]
  4. WbT[c,o] = sum_e g_e WeT[e] PE: 8 accumulating matmuls per c-tile with
                                 lhsT = g_e * I_128 (scaled identity)
  5. y = xT.T @ WbT + 1 x bb     PE: 8 c-tile matmuls + one K=1 bias matmul
                                 per [128-token, 512-out] PSUM group
All host-side work is layout only: transpose/slice/cast so every DMA is
contiguous, then scatter the 8 per-core results into the full outputs.
"""

import numpy as np
import ml_dtypes

import concourse.bass as bass
import concourse.mybir as mybir
import concourse.tile as tile
from concourse import bacc
from concourse.bass import ds, ts
from concourse.bass_utils import run_bass_kernel_spmd
from concourse.masks import make_identity

BF16 = ml_dtypes.bfloat16

B, N, C, O, E = 4, 2048, 1024, 1024, 8
P = 128
NT_C = C // P        # 8 c-tiles (contraction)
NT_N = N // P        # 16 token tiles per tensor
OH = O // 2          # 512 output cols per core
F1 = C // 2          # 512 gate hidden
NT_F = F1 // P       # 4 gate-hidden tiles

_CACHED = {}


def _build_program():
    nc = bacc.Bacc("TRN2", target_bir_lowering=False, debug=False)
    f32 = mybir.dt.float32
    bf16 = mybir.dt.bfloat16

    # DRAM I/O — shapes mirror the SBUF layouts exactly (host pre-arranges).
    xt_d = nc.dram_tensor("xt", [NT_C, P, N], bf16, kind="ExternalInput").ap()
    xirt_d = nc.dram_tensor("xirt", [NT_C, P, N], bf16, kind="ExternalInput").ap()
    wet_d = nc.dram_tensor("wet", [NT_C, P, E, OH], bf16, kind="ExternalInput").ap()
    gw1t_d = nc.dram_tensor("gw1t", [P, NT_C, F1], bf16, kind="ExternalInput").ap()
    gw2t_d = nc.dram_tensor("gw2t", [P, NT_F, E], bf16, kind="ExternalInput").ap()
    gb1t_d = nc.dram_tensor("gb1t", [P, NT_F], f32, kind="ExternalInput").ap()
    gb2_d = nc.dram_tensor("gb2", [1, E], f32, kind="ExternalInput").ap()
    beh_d = nc.dram_tensor("beh", [E, OH], bf16, kind="ExternalInput").ap()
    y_d = nc.dram_tensor("y", [2, NT_N, P, OH], f32, kind="ExternalOutput").ap()

    with tile.TileContext(nc) as tc:
        with (
            tc.tile_pool(name="sb", bufs=1) as sb,
            tc.tile_pool(name="ypool", bufs=4) as ypool,
            tc.tile_pool(name="gwps", bufs=2, space="PSUM") as gwps,
            tc.tile_pool(name="yps", bufs=6, space="PSUM") as yps,
        ):
            # ---- constants ----
            ident = sb.tile([P, P], f32)
            make_identity(nc, ident)
            ones_bf = sb.tile([1, P], bf16)
            nc.any.memset(ones_bf, 1.0)
            ones_f32 = sb.tile([1, P], f32)
            nc.any.memset(ones_f32, 1.0)

            # ---- persistent SBUF tiles + input DMAs ----
            gw1t = sb.tile([P, NT_C * F1], bf16)
            nc.sync.dma_start(gw1t, gw1t_d.rearrange("p t f -> p (t f)"))
            gw2t = sb.tile([P, NT_F * E], bf16)
            nc.sync.dma_start(gw2t, gw2t_d.rearrange("p t e -> p (t e)"))
            gb1t = sb.tile([P, NT_F], f32)
            nc.sync.dma_start(gb1t, gb1t_d)
            gb2 = sb.tile([1, E], f32)
            nc.sync.dma_start(gb2, gb2_d)
            beh = sb.tile([E, OH], bf16)
            nc.sync.dma_start(beh, beh_d)

            xt = sb.tile([P, NT_C * N], bf16)
            for t in range(NT_C):
                nc.sync.dma_start(xt[:, ts(t, N)], xt_d[t])
            wet = sb.tile([P, NT_C * E * OH], bf16)
            for t in range(NT_C):
                nc.sync.dma_start(
                    wet[:, ds(t * E * OH, E * OH)],
                    wet_d[t].rearrange("p e o -> p (e o)"),
                )
            xirt = sb.tile([P, NT_C * N], bf16)
            for t in range(NT_C):
                nc.sync.dma_start(xirt[:, ts(t, N)], xirt_d[t])

            # ---- 1. gx = mean_n x[b]  (free-axis reduce per c-tile) ----
            gxs = sb.tile([P, NT_C], f32)
            for t in range(NT_C):
                nc.vector.reduce_sum(
                    gxs[:, ds(t, 1)], xt[:, ts(t, N)], axis=mybir.AxisListType.X
                )
            gxb = sb.tile([P, NT_C], bf16)
            nc.scalar.activation(
                gxb, gxs, mybir.ActivationFunctionType.Copy, scale=1.0 / N
            )

            # ---- 2. hT = relu(gw1 @ gx + gb1), hidden on partitions ----
            hts = sb.tile([P, NT_F], bf16)
            for ft in range(NT_F):
                hps = gwps.tile([P, 1], f32, tag="gp", name="hps")
                for t in range(NT_C):
                    nc.tensor.matmul(
                        hps,
                        lhsT=gw1t[:, ds(t * F1 + ft * P, P)],
                        rhs=gxb[:, ds(t, 1)],
                        start=(t == 0),
                        stop=(t == NT_C - 1),
                    )
                nc.scalar.activation(
                    hts[:, ds(ft, 1)],
                    hps,
                    mybir.ActivationFunctionType.Relu,
                    bias=gb1t[:, ds(ft, 1)],
                )

            # ---- 3. logits = gw2 @ h + gb2 -> softmax -> g [1, E] ----
            lps = gwps.tile([1, E], f32, tag="gp", name="lps")
            for ft in range(NT_F):
                nc.tensor.matmul(
                    lps,
                    lhsT=hts[:, ds(ft, 1)],
                    rhs=gw2t[:, ts(ft, E)],
                    start=(ft == 0),
                    stop=(ft == NT_F - 1),
                )
            lgs = sb.tile([1, E], f32)
            nc.vector.tensor_add(lgs, lps, gb2)
            mx = sb.tile([1, 1], f32)
            nc.vector.reduce_max(mx, lgs, axis=mybir.AxisListType.X)
            expv = sb.tile([1, E], f32)
            nc.vector.tensor_scalar(
                expv, lgs, mx, None, op0=mybir.AluOpType.subtract
            )
            nc.scalar.activation(expv, expv, mybir.ActivationFunctionType.Exp)
            sm = sb.tile([1, 1], f32)
            nc.vector.reduce_sum(sm, expv, axis=mybir.AxisListType.X)
            rc = sb.tile([1, 1], f32)
            nc.vector.reciprocal(rc, sm)
            gv = sb.tile([1, E], f32)
            nc.vector.tensor_scalar(gv, expv, rc, None, op0=mybir.AluOpType.mult)

            # g on partitions (for the bias matvec): gT = transpose(g)
            gtp = gwps.tile([E, 1], f32, tag="gp", name="gtp")
            nc.tensor.transpose(gtp, gv, ident[0:1, 0:1])
            gtb = sb.tile([E, 1], bf16)
            nc.vector.tensor_copy(gtb, gtp)

            # g broadcast across partitions: [128, E] = ones.T @ g  (K=1)
            gbp = gwps.tile([P, E], f32, tag="gp", name="gbp")
            nc.tensor.matmul(gbp, lhsT=ones_f32, rhs=gv, start=True, stop=True)
            gbs = sb.tile([P, E], f32)
            nc.vector.tensor_copy(gbs, gbp)

            # gI[e] = g_e * I  (stationary operands for the WbT build)
            gis = sb.tile([P, E * P], bf16)
            for e in range(E):
                nc.vector.tensor_scalar(
                    gis[:, ts(e, P)], ident, gbs[:, ds(e, 1)], None,
                    op0=mybir.AluOpType.mult,
                )

            # bb = g @ be_half  -> [1, OH]
            bbp = gwps.tile([1, OH], f32, tag="gp", name="bbp")
            nc.tensor.matmul(bbp, lhsT=gtb, rhs=beh, start=True, stop=True)
            bbs = sb.tile([1, OH], bf16)
            nc.scalar.activation(bbs, bbp, mybir.ActivationFunctionType.Copy)

            # ---- 4. WbT[c-tile] = sum_e g_e WeT[e]  (PE, PSUM accumulate) ----
            wbts = sb.tile([P, NT_C * OH], bf16)
            for t in range(NT_C):
                wp = gwps.tile([P, OH], f32, tag="gp", name="wp")
                for e in range(E):
                    nc.tensor.matmul(
                        wp,
                        lhsT=gis[:, ts(e, P)],
                        rhs=wet[:, ds((t * E + e) * OH, OH)],
                        start=(e == 0),
                        stop=(e == E - 1),
                    )
                nc.scalar.activation(
                    wbts[:, ts(t, OH)], wp, mybir.ActivationFunctionType.Copy
                )

            # ---- 5. y = xT.T @ WbT + ones.T @ bb ----
            for ti, src in ((0, xt), (1, xirt)):
                for nt in range(NT_N):
                    yp = yps.tile([P, OH], f32, tag="yp", name="yp")
                    for t in range(NT_C):
                        nc.tensor.matmul(
                            yp,
                            lhsT=src[:, ds(t * N + nt * P, P)],
                            rhs=wbts[:, ts(t, OH)],
                            start=(t == 0),
                            stop=False,
                        )
                    nc.tensor.matmul(yp, lhsT=ones_bf, rhs=bbs, start=False, stop=True)
                    ys = ypool.tile([P, OH], f32, tag="ys", name="ys")
                    if nt % 2 == 0:
                        nc.scalar.activation(
                            ys, yp, mybir.ActivationFunctionType.Copy
                        )
                    else:
                        nc.vector.tensor_copy(ys, yp)
                    nc.sync.dma_start(y_d[ti, nt], ys)

    nc.compile()
    return nc


def _prep_inputs(x, x_ir, We, be, gw1, gb1, gw2, gb2):
    """Host-side layout shuffling into per-core contiguous DMA images."""
    # xT per batch: [C, N] -> [NT_C, P, N]
    def tokens_t(a, b):  # a: [B, N, C]
        return np.ascontiguousarray(
            a[b].T.reshape(NT_C, P, N)
        ).astype(BF16)

    gw1t = np.ascontiguousarray(
        gw1.T.reshape(NT_C, P, F1).transpose(1, 0, 2)
    ).astype(BF16)                                     # [P, NT_C, F1]
    gw2t = np.ascontiguousarray(
        gw2.T.reshape(NT_F, P, E).transpose(1, 0, 2)
    ).astype(BF16)                                     # [P, NT_F, E]
    gb1t = np.ascontiguousarray(
        gb1.reshape(NT_F, P).T
    ).astype(np.float32)                               # [P, NT_F]
    gb2v = gb2.reshape(1, E).astype(np.float32)

    in_maps = []
    for k in range(8):
        b, h = k % 4, k // 4
        # WeT half: [E, OH, C] slice -> [C, E, OH] -> [NT_C, P, E, OH]
        weh = We[:, h * OH:(h + 1) * OH, :]            # [E, OH, C]
        wet = np.ascontiguousarray(
            weh.transpose(2, 0, 1).reshape(NT_C, P, E, OH)
        ).astype(BF16)
        in_maps.append({
            "xt": tokens_t(x, b),
            "xirt": tokens_t(x_ir, b),
            "wet": wet,
            "gw1t": gw1t,
            "gw2t": gw2t,
            "gb1t": gb1t,
            "gb2": gb2v,
            "beh": np.ascontiguousarray(
                be[:, h * OH:(h + 1) * OH]
            ).astype(BF16),
        })
    return in_maps


def kernel(x, x_ir, We, be, gw1, gb1, gw2, gb2, _trace=False):
    if "nc" not in _CACHED:
        _CACHED["nc"] = _build_program()
    nc = _CACHED["nc"]

    in_maps = _prep_inputs(
        np.asarray(x), np.asarray(x_ir), np.asarray(We), np.asarray(be),
        np.asarray(gw1), np.asarray(gb1), np.asarray(gw2), np.asarray(gb2),
    )
    res = run_bass_kernel_spmd(nc, in_maps, core_ids=list(range(8)), trace=_trace)
    _CACHED["last_result"] = res

    out = np.empty((2, B, N, C), np.float32)
    for k in range(8):
        b, h = k % 4, k // 4
        y = res.results[k]["y"]                        # [2, NT_N, P, OH]
        out[:, b, :, h * OH:(h + 1) * OH] = y.reshape(2, N, OH)
    return out[0], out[1]
